# revision 3
# baseline (speedup 1.0000x reference)
"""Cross-attention Trainium2 kernel (8-core data-parallel over batch).

Per-core computation (one batch element per NeuronCore):
  q = x @ Wq; k = ctx @ Wk; v = ctx @ Wv
  attn = softmax((q k^T) / sqrt(dh)); out = attn @ v; y = out @ Wo + bo

Everything on-chip is kept in "transposed" orientation (feature dim on
partitions, tokens on the free dim) so every matmul streams N=512-wide
moving operands:
  xT   [qd, tok]    via DMA XBAR transposes of natural x tiles (bf16),
                    off all compute engines
  qT   [inner, tok] = Wq_chunk^T @ xT            (bf16 in, fp32 accum)
  sT   [ctx, tok]   = k_hT^T @ q_hT              (head pairs at partition
                                                  bases 0/64 issued back-to-
                                                  back into one 2-bank tile
                                                  so they pack on disjoint
                                                  PE row-groups)
  e    [ctx, tok]   = exp(sT / 8)                (ACT; max-subtraction not
                                                  needed: |scores/8| <~ 6)
  r    [128, tok]   = per-head column sums of e via two col-tiled ones[77,64]
                      matmuls into one bank (partitions 0-63 = head0 sum,
                      64-127 = head1 sum, pre-broadcast)
  outT [dh, tok]    = v_h^T @ e                  (unnormalized, col-tiled
                                                  head pairs pack)
  outT_norm         = outT * (1/r)               (DVE)
  y    [tok, qd]    = outT^T @ Wo + bo           (natural orientation)

DMA queue layout (each engine's DGE ring is in-order, rings run parallel):
  gpsimd SWDGE (casting): ctx, x0, x1, Wv, Wo, x2..x7  (fp32->bf16 in-DMA)
  sync HWDGE:             Wq, Wk (fp32, cast on DVE), bo, y stores
  scalar HWDGE:           XBAR transposes x_g -> xT (16 per group)
y stores are split per 128-token subtile so the tail drains early.
"""

import numpy as np

import concourse.bass as bass
import concourse.tile as tile
from concourse import bacc, mybir
from concourse.bass_utils import run_bass_kernel_spmd
from concourse.masks import make_identity

F32 = mybir.dt.float32
BF16 = mybir.dt.bfloat16

B, N, M = 8, 4096, 77
QD, CD, H, DH = 512, 768, 8, 64
INNER = H * DH  # 512
P = 128
S = 512  # token group size
NQC = QD // P  # 4 qd chunks
NCC = CD // P  # 6 cd chunks
NIC = INNER // P  # 4 inner chunks
NTS = S // P  # 4 token sub-tiles per group
SCALE = DH ** -0.5
MP = 128  # context length padded to full partition width (zeros are inert)


def build_kernel(groups: int = N // S):
    nc = bacc.Bacc(None, target_bir_lowering=False, debug=False)

    x_d = nc.dram_tensor("x", [N, QD], F32, kind="ExternalInput")
    ctx_d = nc.dram_tensor("context", [M, CD], F32, kind="ExternalInput")
    wq_d = nc.dram_tensor("Wq", [QD, INNER], F32, kind="ExternalInput")
    wk_d = nc.dram_tensor("Wk", [CD, INNER], F32, kind="ExternalInput")
    wv_d = nc.dram_tensor("Wv", [CD, INNER], F32, kind="ExternalInput")
    wo_d = nc.dram_tensor("Wo", [INNER, QD], F32, kind="ExternalInput")
    bo_d = nc.dram_tensor("bo", [QD], F32, kind="ExternalInput")
    y_d = nc.dram_tensor("y", [N, QD], F32, kind="ExternalOutput")

    from contextlib import ExitStack

    with tile.TileContext(nc) as tc, ExitStack() as st:
        consts = st.enter_context(tc.tile_pool(name="consts", bufs=1))
        stg = st.enter_context(tc.tile_pool(name="stg", bufs=1))
        kvp = st.enter_context(tc.tile_pool(name="kv", bufs=1))
        xin = st.enter_context(tc.tile_pool(name="xin", bufs=3))
        xtp = st.enter_context(tc.tile_pool(name="xt", bufs=2))
        qtp = st.enter_context(tc.tile_pool(name="qt", bufs=2))
        expp = st.enter_context(tc.tile_pool(name="expp", bufs=2))
        rcp = st.enter_context(tc.tile_pool(name="rcp", bufs=2))
        outp = st.enter_context(tc.tile_pool(name="outp", bufs=2))
        yp = st.enter_context(tc.tile_pool(name="yp", bufs=2))

        # PSUM budget: 8 banks total (2 + 2*2 + 2).
        ps_qf = st.enter_context(tc.tile_pool(name="ps_qf", bufs=2, space="PSUM"))
        ps_s = st.enter_context(tc.tile_pool(name="ps_s", bufs=2, space="PSUM"))
        ps_ro = st.enter_context(tc.tile_pool(name="ps_ro", bufs=2, space="PSUM"))

        identity = consts.tile([P, P], BF16)
        make_identity(nc, identity)

        # ---- gpsimd SWDGE ring: ctx, x0, x1, Wv, Wo, then x2.. ------------------
        ctx_sb = kvp.tile([MP, CD], BF16)
        nc.vector.memset(ctx_sb, 0.0)
        nc.gpsimd.dma_start(out=ctx_sb[:M, :], in_=ctx_d[:, :])

        def load_x(g):
            x_g = xin.tile([P, NTS, QD], BF16)
            nc.gpsimd.dma_start(
                out=x_g,
                in_=x_d[g * S : (g + 1) * S, :].rearrange("(t p) q -> p t q", p=P),
            )
            return x_g

        x_pre = [load_x(0), load_x(1)]

        wv_sb = consts.tile([P, NCC, INNER], BF16)
        nc.gpsimd.dma_start(
            out=wv_sb, in_=wv_d.ap().rearrange("(c p) n -> p c n", p=P)
        )
        wo_sb = consts.tile([P, NIC, QD], BF16)
        nc.gpsimd.dma_start(
            out=wo_sb, in_=wo_d.ap().rearrange("(c p) n -> p c n", p=P)
        )

        # ---- sync ring: Wq, Wk fp32 (DVE casts), bo ------------------------------
        wq32 = stg.tile([P, NQC, INNER], F32)
        nc.sync.dma_start(
            out=wq32, in_=wq_d.ap().rearrange("(c p) n -> p c n", p=P)
        )
        wk32 = stg.tile([P, NCC, INNER], F32)
        nc.sync.dma_start(
            out=wk32, in_=wk_d.ap().rearrange("(c p) n -> p c n", p=P)
        )
        bo_bc = consts.tile([P, QD], F32)
        bo_ap = bo_d.ap()
        nc.sync.dma_start(
            out=bo_bc, in_=bass.AP(bo_ap.tensor, bo_ap.offset, [[0, P], [1, QD]])
        )

        wq_sb = consts.tile([P, NQC, INNER], BF16)
        for c in range(NQC):
            nc.vector.tensor_copy(out=wq_sb[:, c, :], in_=wq32[:, c, :])
        wk_sb = consts.tile([P, NCC, INNER], BF16)
        for c in range(NCC):
            nc.vector.tensor_copy(out=wk_sb[:, c, :], in_=wk32[:, c, :])

        # ones selector for col-tiled rowsums: r[side*64+j, t] = sum_m exp_h[m, t]
        ones77 = consts.tile([M, DH], BF16)
        nc.vector.memset(ones77, 1.0)

        # ---- context projections (tiny) -----------------------------------------
        ctxT = kvp.tile([P, NCC, MP], BF16)
        for cc in range(NCC):
            pt = ps_qf.tile([P, MP], BF16, tag="ps_qf")
            nc.tensor.transpose(
                pt, ctx_sb[:, cc * P : (cc + 1) * P], identity
            )
            nc.vector.tensor_copy(out=ctxT[:, cc, :], in_=pt)

        kT = kvp.tile([P, NIC, MP], BF16)
        for ic in range(NIC):
            pk = ps_qf.tile([P, S], F32, tag="ps_qf")
            for cc in range(NCC):
                nc.tensor.matmul(
                    pk[:, :MP],
                    wk_sb[:, cc, ic * P : (ic + 1) * P],
                    ctxT[:, cc, :],
                    start=(cc == 0),
                    stop=(cc == NCC - 1),
                )
            nc.vector.tensor_copy(out=kT[:, ic, :], in_=pk[:, :MP])

        v_sb = kvp.tile([MP, INNER], BF16)
        pv = ps_qf.tile([MP, INNER], F32, tag="ps_qf")
        for cc in range(NCC):
            nc.tensor.matmul(
                pv,
                ctxT[:, cc, :],
                wv_sb[:, cc, :],
                start=(cc == 0),
                stop=(cc == NCC - 1),
            )
        nc.vector.tensor_copy(out=v_sb, in_=pv)

        # ---- main loop over token groups ----------------------------------------
        # Software-pipelined emission: group g's rowsums / attention-output /
        # final projection are emitted one iteration later, after group g+1's
        # transpose + q-projection block, so their ACT/DVE dependencies have
        # long since resolved by the time the (in-order) PE queue reaches them.

        def emit_front(g):
            x_g = x_pre[g]
            if g + 2 < groups:
                x_pre.append(load_x(g + 2))

            # XBAR transpose x tiles on the scalar HWDGE ring:
            # xT[p, c, ts*128+j] = x[ts*128+j, c*128+p]
            xT = xtp.tile([P, NQC, S], BF16)
            for c in range(NQC):
                for ts in range(NTS):
                    nc.scalar.dma_start(
                        out=xT[:, c, ts * P : (ts + 1) * P],
                        in_=x_g[:, ts, c * P : (c + 1) * P],
                        transpose=True,
                    )

            # qT[inner, tok]
            qT = qtp.tile([P, NIC, S], BF16)
            for ic in range(NIC):
                pq = ps_qf.tile([P, S], F32, tag="ps_qf")
                for c in range(NQC):
                    nc.tensor.matmul(
                        pq,
                        wq_sb[:, c, ic * P : (ic + 1) * P],
                        xT[:, c, :],
                        start=(c == 0),
                        stop=(c == NQC - 1),
                    )
                nc.scalar.copy(out=qT[:, ic, :], in_=pq)

            # scores -> exp per head pair: both sides of a pair go into one
            # 2-bank psum tile so the two matmuls become ready together and
            # issue back-to-back, packing onto disjoint PE row-groups.
            exp_g = expp.tile([MP, H, S], BF16)
            for pp in range(H // 2):
                ps2 = ps_s.tile([P, 2, S], F32, tag="ps_s")
                for side in range(2):
                    par = side * DH
                    nc.tensor.matmul(
                        ps2[:, side, :],
                        kT[par : par + DH, pp, :],
                        qT[par : par + DH, pp, :],
                        start=True,
                        stop=True,
                    )
                for side in range(2):
                    nc.scalar.activation(
                        out=exp_g[:, 2 * pp + side, :],
                        in_=ps2[:, side, :],
                        func=mybir.ActivationFunctionType.Exp,
                        scale=SCALE,
                    )
            return exp_g

        def emit_back(g, exp_g):
            # rowsums, pre-broadcast across 64 partitions per head: two
            # independent col-tiled matmuls into one bank pack on the PE.
            rec_g = rcp.tile([P, H // 2, S], F32)
            for pp in range(H // 2):
                pr = ps_ro.tile([P, S], F32, tag="ps_ro")
                for side in range(2):
                    nc.tensor.matmul(
                        pr[side * DH : (side + 1) * DH, :],
                        ones77,
                        exp_g[:M, 2 * pp + side, :],
                        start=True,
                        stop=True,
                        tile_position=(0, side * DH),
                    )
                nc.vector.reciprocal_approx_fast(out=rec_g[:, pp, :], in_=pr)

            # outT (unnormalized) * (1/r); pair-packed into one bank
            outT = outp.tile([P, NIC, S], BF16)
            for pp in range(H // 2):
                po = ps_ro.tile([P, S], F32, tag="ps_ro")
                for side in range(2):
                    h = 2 * pp + side
                    nc.tensor.matmul(
                        po[side * DH : (side + 1) * DH, :],
                        v_sb[:, h * DH : (h + 1) * DH],
                        exp_g[:, h, :],
                        start=True,
                        stop=True,
                        tile_position=(0, side * DH),
                    )
                nc.vector.tensor_mul(
                    out=outT[:, pp, :], in0=po, in1=rec_g[:, pp, :]
                )

            # final projection + bias; store each 128-token subtile as soon
            # as its bias add lands so the tail drains early
            y_g = yp.tile([P, NTS, QD], F32)
            for ts in range(NTS):
                pf = ps_qf.tile([P, QD], F32, tag="ps_qf")
                for ic in range(NIC):
                    nc.tensor.matmul(
                        pf,
                        outT[:, ic, ts * P : (ts + 1) * P],
                        wo_sb[:, ic, :],
                        start=(ic == 0),
                        stop=(ic == NIC - 1),
                    )
                nc.vector.tensor_add(out=y_g[:, ts, :], in0=pf, in1=bo_bc)
                tok = slice(g * S + ts * P, g * S + (ts + 1) * P)
                nc.sync.dma_start(out=y_d[tok, :], in_=y_g[:, ts, :])

        pending = None
        for g in range(groups):
            exp_g = emit_front(g)
            if pending is not None:
                emit_back(pending[0], pending[1])
            pending = (g, exp_g)
        emit_back(pending[0], pending[1])

    nc.compile()
    return nc


_CACHE = {}


def _get_nc():
    if "nc" not in _CACHE:
        _CACHE["nc"] = build_kernel()
    return _CACHE["nc"]


def run(inputs, trace=False, **kw):
    nc = _get_nc()
    in_maps = []
    for i in range(B):
        m = {
            "x": np.asarray(inputs["x"][i], dtype=np.float32),
            "context": np.asarray(inputs["context"][i], dtype=np.float32),
            "Wq": np.asarray(inputs["Wq"], dtype=np.float32),
            "Wk": np.asarray(inputs["Wk"], dtype=np.float32),
            "Wv": np.asarray(inputs["Wv"], dtype=np.float32),
            "Wo": np.asarray(inputs["Wo"], dtype=np.float32),
            "bo": np.asarray(inputs["bo"], dtype=np.float32),
        }
        in_maps.append(m)
    res = run_bass_kernel_spmd(nc, in_maps, list(range(B)), trace=trace, **kw)
    out = np.stack([res.results[i]["y"] for i in range(B)], axis=0)
    return out, res


def kernel(**inputs):
    out, _ = run(inputs)
    return out


# revision 8
# speedup vs baseline: 2.1180x; 2.1180x over previous
"""Cross-attention Trainium2 kernel (8-core data-parallel over batch).

Per-core computation (one batch element per NeuronCore):
  q = x @ Wq; k = ctx @ Wk; v = ctx @ Wv
  attn = softmax((q k^T) / sqrt(dh)); out = attn @ v; y = out @ Wo + bo

Everything on-chip is kept in "transposed" orientation (feature dim on
partitions, tokens on the free dim) so every matmul streams N=512-wide
moving operands:
  xT   [qd, tok]    via PE transposes of natural x tiles (bf16)
  qT   [inner, tok] = Wq_chunk^T @ xT            (bf16 in, fp32 accum)
  sT   [ctx, tok]   = k_hT^T @ q_hT              (head pairs at partition
                                                  bases 0/64 issued back-to-
                                                  back into one 2-bank tile
                                                  so they pack on disjoint
                                                  PE row-groups)
  e    [ctx, tok]   = exp(sT / 8)                (ACT; max-subtraction not
                                                  needed: |scores/8| <~ 6)
  r    [128, tok]   = per-head column sums of e via two col-tiled ones[77,64]
                      matmuls into one bank (partitions 0-63 = head0 sum,
                      64-127 = head1 sum, pre-broadcast)
  outT [dh, tok]    = v_h^T @ e                  (unnormalized, col-tiled
                                                  head pairs pack)
  outT_norm         = outT * (1/r)               (DVE)
  y    [tok, qd]    = outT^T @ Wo + bo           (natural orientation)

DMA queue layout (each engine's DGE ring is in-order, rings run parallel):
  gpsimd SWDGE (casting): ctx, x0, x1, Wv, Wo, x2..x7  (fp32->bf16 in-DMA)
  sync HWDGE:             Wq, Wk (fp32, cast on DVE), bo, y stores
y stores are split per 128-token subtile so the tail drains early.
"""

import numpy as np

import concourse.bass as bass
import concourse.tile as tile
from concourse import bacc, mybir
from concourse.bass_utils import run_bass_kernel_spmd
from concourse.masks import make_identity

F32 = mybir.dt.float32
BF16 = mybir.dt.bfloat16

B, N, M = 8, 4096, 77
QD, CD, H, DH = 512, 768, 8, 64
INNER = H * DH  # 512
P = 128
S = 512  # token group size
NQC = QD // P  # 4 qd chunks
NCC = CD // P  # 6 cd chunks
NIC = INNER // P  # 4 inner chunks
NTS = S // P  # 4 token sub-tiles per group
SCALE = DH ** -0.5
MP = 128  # context length padded to full partition width (zeros are inert)


def build_kernel(groups: int = N // S):
    nc = bacc.Bacc(None, target_bir_lowering=False, debug=False)

    x_d = nc.dram_tensor("x", [N, QD], F32, kind="ExternalInput")
    ctx_d = nc.dram_tensor("context", [M, CD], F32, kind="ExternalInput")
    wq_d = nc.dram_tensor("Wq", [QD, INNER], F32, kind="ExternalInput")
    wk_d = nc.dram_tensor("Wk", [CD, INNER], F32, kind="ExternalInput")
    wv_d = nc.dram_tensor("Wv", [CD, INNER], F32, kind="ExternalInput")
    wo_d = nc.dram_tensor("Wo", [INNER, QD], F32, kind="ExternalInput")
    bo_d = nc.dram_tensor("bo", [QD], F32, kind="ExternalInput")
    y_d = nc.dram_tensor("y", [N, QD], F32, kind="ExternalOutput")

    from contextlib import ExitStack

    with tile.TileContext(nc) as tc, ExitStack() as st:
        consts = st.enter_context(tc.tile_pool(name="consts", bufs=1))
        stg = st.enter_context(tc.tile_pool(name="stg", bufs=1))
        kvp = st.enter_context(tc.tile_pool(name="kv", bufs=1))
        xin = st.enter_context(tc.tile_pool(name="xin", bufs=3))
        xtp = st.enter_context(tc.tile_pool(name="xt", bufs=2))
        qtp = st.enter_context(tc.tile_pool(name="qt", bufs=2))
        expp = st.enter_context(tc.tile_pool(name="expp", bufs=2))
        rcp = st.enter_context(tc.tile_pool(name="rcp", bufs=2))
        outp = st.enter_context(tc.tile_pool(name="outp", bufs=2))
        yp = st.enter_context(tc.tile_pool(name="yp", bufs=2))

        # PSUM budget: 8 banks total (2 + 2 + 2 + 2).
        ps_tr = st.enter_context(tc.tile_pool(name="ps_tr", bufs=2, space="PSUM"))
        ps_qf = st.enter_context(tc.tile_pool(name="ps_qf", bufs=2, space="PSUM"))
        ps_s = st.enter_context(tc.tile_pool(name="ps_s", bufs=1, space="PSUM"))
        ps_ro = st.enter_context(tc.tile_pool(name="ps_ro", bufs=2, space="PSUM"))

        identity = consts.tile([P, P], BF16)
        make_identity(nc, identity)

        # ---- gpsimd SWDGE ring: ctx, x0, x1, Wv, Wo, then x2.. ------------------
        ctx_sb = kvp.tile([MP, CD], BF16)
        nc.vector.memset(ctx_sb, 0.0)
        nc.gpsimd.dma_start(out=ctx_sb[:M, :], in_=ctx_d[:, :])

        def load_x(g):
            x_g = xin.tile([P, NTS, QD], BF16)
            nc.gpsimd.dma_start(
                out=x_g,
                in_=x_d[g * S : (g + 1) * S, :].rearrange("(t p) q -> p t q", p=P),
            )
            return x_g

        x_pre = [load_x(0), load_x(1)]

        wv_sb = consts.tile([P, NCC, INNER], BF16)
        nc.gpsimd.dma_start(
            out=wv_sb, in_=wv_d.ap().rearrange("(c p) n -> p c n", p=P)
        )
        wo_sb = consts.tile([P, NIC, QD], BF16)
        nc.gpsimd.dma_start(
            out=wo_sb, in_=wo_d.ap().rearrange("(c p) n -> p c n", p=P)
        )

        # ---- sync ring: Wq, Wk fp32 (DVE casts), bo ------------------------------
        wq32 = stg.tile([P, NQC, INNER], F32)
        nc.sync.dma_start(
            out=wq32, in_=wq_d.ap().rearrange("(c p) n -> p c n", p=P)
        )
        wk32 = stg.tile([P, NCC, INNER], F32)
        nc.sync.dma_start(
            out=wk32, in_=wk_d.ap().rearrange("(c p) n -> p c n", p=P)
        )
        bo_bc = consts.tile([P, QD], F32)
        bo_ap = bo_d.ap()
        nc.sync.dma_start(
            out=bo_bc, in_=bass.AP(bo_ap.tensor, bo_ap.offset, [[0, P], [1, QD]])
        )

        wq_sb = consts.tile([P, NQC, INNER], BF16)
        for c in range(NQC):
            nc.vector.tensor_copy(out=wq_sb[:, c, :], in_=wq32[:, c, :])
        wk_sb = consts.tile([P, NCC, INNER], BF16)
        for c in range(NCC):
            nc.vector.tensor_copy(out=wk_sb[:, c, :], in_=wk32[:, c, :])

        # ones selector for col-tiled rowsums: r[side*64+j, t] = sum_m exp_h[m, t]
        ones77 = consts.tile([M, DH], BF16)
        nc.vector.memset(ones77, 1.0)

        # ---- context projections (tiny) -----------------------------------------
        ctxT = kvp.tile([P, NCC, MP], BF16)
        for cc in range(NCC):
            pt = ps_tr.tile([P, MP], BF16, tag="ps_tr")
            nc.tensor.transpose(
                pt, ctx_sb[:, cc * P : (cc + 1) * P], identity
            )
            nc.vector.tensor_copy(out=ctxT[:, cc, :], in_=pt)

        kT = kvp.tile([P, NIC, MP], BF16)
        for ic in range(NIC):
            pk = ps_qf.tile([P, S], F32, tag="ps_qf")
            for cc in range(NCC):
                nc.tensor.matmul(
                    pk[:, :MP],
                    wk_sb[:, cc, ic * P : (ic + 1) * P],
                    ctxT[:, cc, :],
                    start=(cc == 0),
                    stop=(cc == NCC - 1),
                )
            nc.vector.tensor_copy(out=kT[:, ic, :], in_=pk[:, :MP])

        v_sb = kvp.tile([MP, INNER], BF16)
        pv = ps_qf.tile([MP, INNER], F32, tag="ps_qf")
        for cc in range(NCC):
            nc.tensor.matmul(
                pv,
                ctxT[:, cc, :],
                wv_sb[:, cc, :],
                start=(cc == 0),
                stop=(cc == NCC - 1),
            )
        nc.vector.tensor_copy(out=v_sb, in_=pv)

        # ---- main loop over token groups ----------------------------------------
        # Software-pipelined emission: group g's rowsums / attention-output /
        # final projection are emitted one iteration later, after group g+1's
        # transpose + q-projection block, so their ACT/DVE dependencies have
        # long since resolved by the time the (in-order) PE queue reaches them.

        def emit_front(g):
            x_g = x_pre[g]
            if g + 2 < groups:
                x_pre.append(load_x(g + 2))

            # transpose x tiles: xT[p, c, t*128+j] = x[t*128+..., c*128+p];
            # 4 PE transposes land in one psum bank, one DVE copy per chunk
            xT = xtp.tile([P, NQC, S], BF16)
            for c in range(NQC):
                pt = ps_tr.tile([P, S], BF16, tag="ps_tr")
                for ts in range(NTS):
                    nc.tensor.transpose(
                        pt[:, ts * P : (ts + 1) * P],
                        x_g[:, ts, c * P : (c + 1) * P],
                        identity,
                    )
                nc.vector.tensor_copy(out=xT[:, c, :], in_=pt)

            # qT[inner, tok]
            qT = qtp.tile([P, NIC, S], BF16)
            for ic in range(NIC):
                pq = ps_qf.tile([P, S], F32, tag="ps_qf")
                for c in range(NQC):
                    nc.tensor.matmul(
                        pq,
                        wq_sb[:, c, ic * P : (ic + 1) * P],
                        xT[:, c, :],
                        start=(c == 0),
                        stop=(c == NQC - 1),
                    )
                nc.scalar.copy(out=qT[:, ic, :], in_=pq)

            # scores -> exp per head pair: both sides of a pair go into one
            # 2-bank psum tile so the two matmuls become ready together and
            # issue back-to-back, packing onto disjoint PE row-groups.
            exp_g = expp.tile([MP, H, S], BF16)
            for pp in range(H // 2):
                ps2 = ps_s.tile([P, 2, S], F32, tag="ps_s")
                for side in range(2):
                    par = side * DH
                    nc.tensor.matmul(
                        ps2[:, side, :],
                        kT[par : par + DH, pp, :],
                        qT[par : par + DH, pp, :],
                        start=True,
                        stop=True,
                    )
                for side in range(2):
                    nc.scalar.activation(
                        out=exp_g[:, 2 * pp + side, :],
                        in_=ps2[:, side, :],
                        func=mybir.ActivationFunctionType.Exp,
                        scale=SCALE,
                    )
            return exp_g

        def emit_back(g, exp_g):
            # rowsums, pre-broadcast across 64 partitions per head: two
            # independent col-tiled matmuls into one bank pack on the PE.
            rec_g = rcp.tile([P, H // 2, S], F32)
            for pp in range(H // 2):
                pr = ps_ro.tile([P, S], F32, tag="ps_ro")
                for side in range(2):
                    nc.tensor.matmul(
                        pr[side * DH : (side + 1) * DH, :],
                        ones77,
                        exp_g[:M, 2 * pp + side, :],
                        start=True,
                        stop=True,
                        tile_position=(0, side * DH),
                    )
                nc.vector.reciprocal_approx_fast(out=rec_g[:, pp, :], in_=pr)

            # outT (unnormalized) * (1/r); pair-packed into one bank
            outT = outp.tile([P, NIC, S], BF16)
            for pp in range(H // 2):
                po = ps_ro.tile([P, S], F32, tag="ps_ro")
                for side in range(2):
                    h = 2 * pp + side
                    nc.tensor.matmul(
                        po[side * DH : (side + 1) * DH, :],
                        v_sb[:, h * DH : (h + 1) * DH],
                        exp_g[:, h, :],
                        start=True,
                        stop=True,
                        tile_position=(0, side * DH),
                    )
                nc.vector.tensor_mul(
                    out=outT[:, pp, :], in0=po, in1=rec_g[:, pp, :]
                )

            # final projection + bias; store each 128-token subtile as soon
            # as its bias add lands so the tail drains early
            y_g = yp.tile([P, NTS, QD], F32)
            for ts in range(NTS):
                pf = ps_qf.tile([P, QD], F32, tag="ps_qf")
                for ic in range(NIC):
                    nc.tensor.matmul(
                        pf,
                        outT[:, ic, ts * P : (ts + 1) * P],
                        wo_sb[:, ic, :],
                        start=(ic == 0),
                        stop=(ic == NIC - 1),
                    )
                nc.vector.tensor_add(out=y_g[:, ts, :], in0=pf, in1=bo_bc)
                tok = slice(g * S + ts * P, g * S + (ts + 1) * P)
                nc.sync.dma_start(out=y_d[tok, :], in_=y_g[:, ts, :])

        pending = None
        for g in range(groups):
            exp_g = emit_front(g)
            if pending is not None:
                emit_back(pending[0], pending[1])
            pending = (g, exp_g)
        emit_back(pending[0], pending[1])

    nc.compile()
    return nc


_CACHE = {}


def _get_nc():
    if "nc" not in _CACHE:
        _CACHE["nc"] = build_kernel()
    return _CACHE["nc"]


def run(inputs, trace=False, **kw):
    nc = _get_nc()
    in_maps = []
    for i in range(B):
        m = {
            "x": np.asarray(inputs["x"][i], dtype=np.float32),
            "context": np.asarray(inputs["context"][i], dtype=np.float32),
            "Wq": np.asarray(inputs["Wq"], dtype=np.float32),
            "Wk": np.asarray(inputs["Wk"], dtype=np.float32),
            "Wv": np.asarray(inputs["Wv"], dtype=np.float32),
            "Wo": np.asarray(inputs["Wo"], dtype=np.float32),
            "bo": np.asarray(inputs["bo"], dtype=np.float32),
        }
        in_maps.append(m)
    res = run_bass_kernel_spmd(nc, in_maps, list(range(B)), trace=trace, **kw)
    out = np.stack([res.results[i]["y"] for i in range(B)], axis=0)
    return out, res


def kernel(**inputs):
    out, _ = run(inputs)
    return out


# revision 11
# speedup vs baseline: 2.1348x; 1.0079x over previous
"""Cross-attention Trainium2 kernel (8-core data-parallel over batch).

Per-core computation (one batch element per NeuronCore):
  q = x @ Wq; k = ctx @ Wk; v = ctx @ Wv
  attn = softmax((q k^T) / sqrt(dh)); out = attn @ v; y = out @ Wo + bo

Everything on-chip is kept in "transposed" orientation (feature dim on
partitions, tokens on the free dim) so every matmul streams N=512-wide
moving operands:
  xT   [qd, tok]    via PE transposes of natural x tiles (bf16)
  qT   [inner, tok] = Wq_chunk^T @ xT            (bf16 in, fp32 accum)
  sT   [ctx, tok]   = k_hT^T @ q_hT              (head pairs at partition
                                                  bases 0/64 issued back-to-
                                                  back into one 2-bank tile
                                                  so they pack on disjoint
                                                  PE row-groups)
  e    [ctx, tok]   = exp(sT / 8)                (ACT; max-subtraction not
                                                  needed: |scores/8| <~ 6)
  r    [128, tok]   = per-head column sums of e via two col-tiled ones[77,64]
                      matmuls into one bank (partitions 0-63 = head0 sum,
                      64-127 = head1 sum, pre-broadcast)
  outT [dh, tok]    = v_h^T @ e                  (unnormalized, col-tiled
                                                  head pairs pack)
  outT_norm         = outT * (1/r)               (DVE)
  y    [tok, qd]    = outT^T @ Wo + bo           (natural orientation)

DMA: HBM bandwidth is shared across rings, so all input loads go on the
single gpsimd SWDGE ring (which casts fp32->bf16 in-DMA) serialized in
first-need order: ctx, x0, Wq, Wk, x1, Wv, Wo, bo, x2, x3, ...
y stores run on the sync HWDGE ring, split per 128-token subtile so the
tail drains early.
"""

import numpy as np

import concourse.bass as bass
import concourse.tile as tile
from concourse import bacc, mybir
from concourse.bass_utils import run_bass_kernel_spmd
from concourse.masks import make_identity

F32 = mybir.dt.float32
BF16 = mybir.dt.bfloat16

B, N, M = 8, 4096, 77
QD, CD, H, DH = 512, 768, 8, 64
INNER = H * DH  # 512
P = 128
S = 512  # token group size
NQC = QD // P  # 4 qd chunks
NCC = CD // P  # 6 cd chunks
NIC = INNER // P  # 4 inner chunks
NTS = S // P  # 4 token sub-tiles per group
SCALE = DH ** -0.5
MP = 128  # context length padded to full partition width (zeros are inert)


def build_kernel(groups: int = N // S):
    nc = bacc.Bacc(None, target_bir_lowering=False, debug=False)

    x_d = nc.dram_tensor("x", [N, QD], F32, kind="ExternalInput")
    ctx_d = nc.dram_tensor("context", [M, CD], F32, kind="ExternalInput")
    wq_d = nc.dram_tensor("Wq", [QD, INNER], F32, kind="ExternalInput")
    wk_d = nc.dram_tensor("Wk", [CD, INNER], F32, kind="ExternalInput")
    wv_d = nc.dram_tensor("Wv", [CD, INNER], F32, kind="ExternalInput")
    wo_d = nc.dram_tensor("Wo", [INNER, QD], F32, kind="ExternalInput")
    bo_d = nc.dram_tensor("bo", [QD], F32, kind="ExternalInput")
    y_d = nc.dram_tensor("y", [N, QD], F32, kind="ExternalOutput")

    from contextlib import ExitStack

    with tile.TileContext(nc) as tc, ExitStack() as st:
        consts = st.enter_context(tc.tile_pool(name="consts", bufs=1))
        kvp = st.enter_context(tc.tile_pool(name="kv", bufs=1))
        xin = st.enter_context(tc.tile_pool(name="xin", bufs=3))
        xtp = st.enter_context(tc.tile_pool(name="xt", bufs=2))
        qtp = st.enter_context(tc.tile_pool(name="qt", bufs=2))
        expp = st.enter_context(tc.tile_pool(name="expp", bufs=2))
        rcp = st.enter_context(tc.tile_pool(name="rcp", bufs=2))
        outp = st.enter_context(tc.tile_pool(name="outp", bufs=2))
        yp = st.enter_context(tc.tile_pool(name="yp", bufs=2))

        # PSUM budget: 8 banks total (2 + 2 + 2 + 2).
        ps_tr = st.enter_context(tc.tile_pool(name="ps_tr", bufs=2, space="PSUM"))
        ps_qf = st.enter_context(tc.tile_pool(name="ps_qf", bufs=2, space="PSUM"))
        ps_s = st.enter_context(tc.tile_pool(name="ps_s", bufs=1, space="PSUM"))
        ps_ro = st.enter_context(tc.tile_pool(name="ps_ro", bufs=2, space="PSUM"))

        identity = consts.tile([P, P], BF16)
        make_identity(nc, identity)

        # ---- gpsimd SWDGE ring: ctx, x0, x1, Wv, Wo, then x2.. ------------------
        ctx_sb = kvp.tile([MP, CD], BF16)
        nc.vector.memset(ctx_sb, 0.0)
        nc.gpsimd.dma_start(out=ctx_sb[:M, :], in_=ctx_d[:, :])

        def load_x(g):
            x_g = xin.tile([P, NTS, QD], BF16)
            nc.gpsimd.dma_start(
                out=x_g,
                in_=x_d[g * S : (g + 1) * S, :].rearrange("(t p) q -> p t q", p=P),
            )
            return x_g

        x_pre = [load_x(0)]

        wq_sb = consts.tile([P, NQC, INNER], BF16)
        nc.gpsimd.dma_start(
            out=wq_sb, in_=wq_d.ap().rearrange("(c p) n -> p c n", p=P)
        )
        wk_sb = consts.tile([P, NCC, INNER], BF16)
        nc.gpsimd.dma_start(
            out=wk_sb, in_=wk_d.ap().rearrange("(c p) n -> p c n", p=P)
        )

        x_pre.append(load_x(1))

        wv_sb = consts.tile([P, NCC, INNER], BF16)
        nc.gpsimd.dma_start(
            out=wv_sb, in_=wv_d.ap().rearrange("(c p) n -> p c n", p=P)
        )
        wo_sb = consts.tile([P, NIC, QD], BF16)
        nc.gpsimd.dma_start(
            out=wo_sb, in_=wo_d.ap().rearrange("(c p) n -> p c n", p=P)
        )
        bo_bc = consts.tile([P, QD], F32)
        bo_ap = bo_d.ap()
        nc.gpsimd.dma_start(
            out=bo_bc, in_=bass.AP(bo_ap.tensor, bo_ap.offset, [[0, P], [1, QD]])
        )

        # ones selector for col-tiled rowsums: r[side*64+j, t] = sum_m exp_h[m, t]
        ones77 = consts.tile([M, DH], BF16)
        nc.vector.memset(ones77, 1.0)

        # ---- context projections (tiny) -----------------------------------------
        ctxT = kvp.tile([P, NCC, MP], BF16)
        for cc in range(NCC):
            pt = ps_tr.tile([P, MP], BF16, tag="ps_tr")
            nc.tensor.transpose(
                pt, ctx_sb[:, cc * P : (cc + 1) * P], identity
            )
            nc.vector.tensor_copy(out=ctxT[:, cc, :], in_=pt)

        kT = kvp.tile([P, NIC, MP], BF16)
        for ic in range(NIC):
            pk = ps_qf.tile([P, S], F32, tag="ps_qf")
            for cc in range(NCC):
                nc.tensor.matmul(
                    pk[:, :MP],
                    wk_sb[:, cc, ic * P : (ic + 1) * P],
                    ctxT[:, cc, :],
                    start=(cc == 0),
                    stop=(cc == NCC - 1),
                )
            nc.vector.tensor_copy(out=kT[:, ic, :], in_=pk[:, :MP])

        v_sb = kvp.tile([MP, INNER], BF16)
        pv = ps_qf.tile([MP, INNER], F32, tag="ps_qf")
        for cc in range(NCC):
            nc.tensor.matmul(
                pv,
                ctxT[:, cc, :],
                wv_sb[:, cc, :],
                start=(cc == 0),
                stop=(cc == NCC - 1),
            )
        nc.vector.tensor_copy(out=v_sb, in_=pv)

        # ---- main loop over token groups ----------------------------------------
        # Software-pipelined emission: group g's rowsums / attention-output /
        # final projection are emitted one iteration later, after group g+1's
        # transpose + q-projection block, so their ACT/DVE dependencies have
        # long since resolved by the time the (in-order) PE queue reaches them.

        def emit_front(g):
            x_g = x_pre[g]
            if g + 2 < groups:
                x_pre.append(load_x(g + 2))

            # transpose x tiles: xT[p, c, t*128+j] = x[t*128+..., c*128+p];
            # 4 PE transposes land in one psum bank, one DVE copy per chunk
            xT = xtp.tile([P, NQC, S], BF16)
            for c in range(NQC):
                pt = ps_tr.tile([P, S], BF16, tag="ps_tr")
                for ts in range(NTS):
                    nc.tensor.transpose(
                        pt[:, ts * P : (ts + 1) * P],
                        x_g[:, ts, c * P : (c + 1) * P],
                        identity,
                    )
                nc.vector.tensor_copy(out=xT[:, c, :], in_=pt)

            # qT[inner, tok]
            qT = qtp.tile([P, NIC, S], BF16)
            for ic in range(NIC):
                pq = ps_qf.tile([P, S], F32, tag="ps_qf")
                for c in range(NQC):
                    nc.tensor.matmul(
                        pq,
                        wq_sb[:, c, ic * P : (ic + 1) * P],
                        xT[:, c, :],
                        start=(c == 0),
                        stop=(c == NQC - 1),
                    )
                nc.scalar.copy(out=qT[:, ic, :], in_=pq)

            # scores -> exp per head pair: both sides of a pair go into one
            # 2-bank psum tile so the two matmuls become ready together and
            # issue back-to-back, packing onto disjoint PE row-groups.
            exp_g = expp.tile([MP, H, S], BF16)
            for pp in range(H // 2):
                ps2 = ps_s.tile([P, 2, S], F32, tag="ps_s")
                for side in range(2):
                    par = side * DH
                    nc.tensor.matmul(
                        ps2[:, side, :],
                        kT[par : par + DH, pp, :],
                        qT[par : par + DH, pp, :],
                        start=True,
                        stop=True,
                    )
                for side in range(2):
                    nc.scalar.activation(
                        out=exp_g[:, 2 * pp + side, :],
                        in_=ps2[:, side, :],
                        func=mybir.ActivationFunctionType.Exp,
                        scale=SCALE,
                    )
            return exp_g

        def emit_back(g, exp_g):
            # rowsums, pre-broadcast across 64 partitions per head: two
            # independent col-tiled matmuls into one bank pack on the PE.
            rec_g = rcp.tile([P, H // 2, S], F32)
            for pp in range(H // 2):
                pr = ps_ro.tile([P, S], F32, tag="ps_ro")
                for side in range(2):
                    nc.tensor.matmul(
                        pr[side * DH : (side + 1) * DH, :],
                        ones77,
                        exp_g[:M, 2 * pp + side, :],
                        start=True,
                        stop=True,
                        tile_position=(0, side * DH),
                    )
                nc.vector.reciprocal_approx_fast(out=rec_g[:, pp, :], in_=pr)

            # outT (unnormalized) * (1/r); pair-packed into one bank
            outT = outp.tile([P, NIC, S], BF16)
            for pp in range(H // 2):
                po = ps_ro.tile([P, S], F32, tag="ps_ro")
                for side in range(2):
                    h = 2 * pp + side
                    nc.tensor.matmul(
                        po[side * DH : (side + 1) * DH, :],
                        v_sb[:, h * DH : (h + 1) * DH],
                        exp_g[:, h, :],
                        start=True,
                        stop=True,
                        tile_position=(0, side * DH),
                    )
                nc.vector.tensor_mul(
                    out=outT[:, pp, :], in0=po, in1=rec_g[:, pp, :]
                )

            # final projection + bias; store each 128-token subtile as soon
            # as its bias add lands so the tail drains early
            y_g = yp.tile([P, NTS, QD], F32)
            for ts in range(NTS):
                pf = ps_qf.tile([P, QD], F32, tag="ps_qf")
                for ic in range(NIC):
                    nc.tensor.matmul(
                        pf,
                        outT[:, ic, ts * P : (ts + 1) * P],
                        wo_sb[:, ic, :],
                        start=(ic == 0),
                        stop=(ic == NIC - 1),
                    )
                nc.vector.tensor_add(out=y_g[:, ts, :], in0=pf, in1=bo_bc)
                tok = slice(g * S + ts * P, g * S + (ts + 1) * P)
                nc.sync.dma_start(out=y_d[tok, :], in_=y_g[:, ts, :])

        pending = None
        for g in range(groups):
            exp_g = emit_front(g)
            if pending is not None:
                emit_back(pending[0], pending[1])
            pending = (g, exp_g)
        emit_back(pending[0], pending[1])

    nc.compile()
    return nc


_CACHE = {}


def _get_nc():
    if "nc" not in _CACHE:
        _CACHE["nc"] = build_kernel()
    return _CACHE["nc"]


def run(inputs, trace=False, **kw):
    nc = _get_nc()
    in_maps = []
    for i in range(B):
        m = {
            "x": np.asarray(inputs["x"][i], dtype=np.float32),
            "context": np.asarray(inputs["context"][i], dtype=np.float32),
            "Wq": np.asarray(inputs["Wq"], dtype=np.float32),
            "Wk": np.asarray(inputs["Wk"], dtype=np.float32),
            "Wv": np.asarray(inputs["Wv"], dtype=np.float32),
            "Wo": np.asarray(inputs["Wo"], dtype=np.float32),
            "bo": np.asarray(inputs["bo"], dtype=np.float32),
        }
        in_maps.append(m)
    res = run_bass_kernel_spmd(nc, in_maps, list(range(B)), trace=trace, **kw)
    out = np.stack([res.results[i]["y"] for i in range(B)], axis=0)
    return out, res


def kernel(**inputs):
    out, _ = run(inputs)
    return out
